# revision 26
# baseline (speedup 1.0000x reference)
"""CBOW forward (mean-embed -> linear -> linear -> log_softmax) on 8 trn2 cores.

Vocab-parallel tensor parallelism: each core owns a V/8 = 4000-wide vocab shard
of the input slices, W1 columns, and W2 rows.  Layer-1 partial h^T is
AllReduced in bf16 (32 KB), layer-2 + softmax statistics are computed
shard-locally with a tiny AllGather of per-core sum(exp(logits)).

Key structure (v2 — DMA-roofline oriented):
 - All heavy tensors are pre-packed HOST-SIDE into bf16, already transposed
   into the exact SBUF layouts the matmuls want.  Per-core HBM ingest is
   8.4 MB (X^T 4.2, W1^T 2.1, W2^T 2.05) vs 16.8 MB for fp32 — the kernel
   ingests at HBM rate and everything else hides behind it.
 - X arrives as X^T[v, b*8+i] so the context mean is a DVE group-reduce
   (axis-X over a [128, 8, 64, 8] view) — no PE work, no on-chip transpose.
 - Layer 1 computes h^T directly (W1 chunk stationary per d-half, x-bar^T
   moving), so the post-AllReduce path has zero transposes.
 - b1/8 is folded into every core's partial pre-AllReduce (8 * b1/8 == b1),
   packed as two bf16 columns inside the W1 tensor so no tiny-descriptor
   DMAs ever hit the queues.
 - The HW DMA queue carries ONLY large transfers (>= 0.5 MB weights/X,
   bounces, output); a warmup AllGather triggered off the gpsimd queue at
   t~6us absorbs cross-core launch skew during ingest.
 - No keep-warm matmuls: trace analysis showed they trip the HW activity
   monitor (HAM) power throttle and run layer 2 at ~1/3 clock.
 - A dummy Exp preloads the ACT function table during the AllReduce gap.

Problem shapes (hardcoded): B=64, 2N=8 context slots, V=32000, D=256, fp32 IO.
"""

import numpy as np

import concourse.bacc as bacc
import concourse.mybir as mybir
import concourse.tile as tile
from concourse.bass_utils import run_bass_kernel_spmd

N_CORES = 8
B = 64          # batch
NCTX = 8        # 2N context slots
V = 32000
D = 256
VS = V // N_CORES          # 4000 vocab columns per core
VC = 128                   # v-chunk = one partition block
NVC = 32                   # chunks per core (padded 4000 -> 4096)
VP = NVC * VC              # 4096 padded vocab rows
ROWS = B * NCTX            # 512 = b*8 + i
NSPL = 8                   # layer-2 n-splits
SW = VS // NSPL            # 500 cols per split
F32 = mybir.dt.float32
BF16 = mybir.dt.bfloat16

_cache = {}


def _build():
    nc = bacc.Bacc("TRN2", target_bir_lowering=False, debug=False,
                   num_devices=N_CORES)

    # Host-packed inputs (see _make_in_maps for layouts).
    XT = nc.dram_tensor("xt", [128, NVC, ROWS], BF16, kind="ExternalInput")
    W1B = nc.dram_tensor("w1b", [128, NVC * D + 2], BF16, kind="ExternalInput")
    W2B = nc.dram_tensor("w2b", [128, 2, VS], BF16, kind="ExternalInput")
    B2 = nc.dram_tensor("b2", [1, VS], BF16, kind="ExternalInput")
    OUT = nc.dram_tensor("out", [B, VS], F32, kind="ExternalOutput")

    rg = [list(range(N_CORES))]

    with tile.TileContext(nc) as tc:
        with (
            tc.tile_pool(name="consts", bufs=1) as consts,
            tc.tile_pool(name="xpool", bufs=1) as xpool,
            tc.tile_pool(name="wpool", bufs=1) as wpool,
            tc.tile_pool(name="work", bufs=1) as work,
            tc.tile_pool(name="dram", bufs=1, space="DRAM") as dram,
        ):
            # Warmup collective: absorbs cross-core launch skew, the ncfw
            # wakeup (~18us after first trigger) and the first-collective
            # barrier while ingest runs.  Input is an uninitialized DRAM
            # tile (values never read) so the trigger fires with no DMA
            # dependency at ~7us.
            warm_sb = consts.tile([1, 16], F32)
            nc.vector.memset(warm_sb[:], 0.0)

            ones_sb = consts.tile([1, B], BF16)
            nc.vector.memset(ones_sb[:], 1.0)

            xt_sb = xpool.tile([128, NVC, ROWS], BF16)      # 4.19 MB
            w1_sb = wpool.tile([128, NVC * D + 2], BF16)    # 2.10 MB
            w2_sb = wpool.tile([128, 2, VS], BF16)          # 2.05 MB
            b2_sb = wpool.tile([1, VS], BF16)
            xbar_sb = work.tile([128, NVC, B], F32)
            xbarb_sb = work.tile([128, NVC, B], BF16)
            hT_sb = work.tile([128, 2, B], BF16)
            b1col_sb = work.tile([128, 2], F32)
            dummy_sb = work.tile([1, 16], F32)

            # Ingest + stage 1 (context mean on DVE) + layer 1 (h^T on PE).
            with tc.tile_pool(name="ps1", bufs=1, space="PSUM") as ps1:
                hT0_ps = ps1.tile([128, B], F32, tag="h0")
                hT1_ps = ps1.tile([128, B], F32, tag="h1")
                QC = NVC // 4          # 8 chunks per quarter
                for q in range(4):
                    nc.sync.dma_start(xt_sb[:, q * QC:(q + 1) * QC, :],
                                      XT.ap()[:, q * QC:(q + 1) * QC, :])
                    w1_hi = (q + 1) * QC * D + (2 if q == 3 else 0)
                    nc.sync.dma_start(
                        w1_sb[:, q * QC * D:w1_hi],
                        W1B.ap()[:, q * QC * D:w1_hi])
                    # mean over the 8 context slots: group-reduce innermost 8
                    nc.vector.reduce_sum(
                        xbar_sb[:, q * QC:(q + 1) * QC, :],
                        xt_sb[:, q * QC:(q + 1) * QC, :].rearrange(
                            "p c (b i) -> p c b i", i=NCTX),
                        axis=mybir.AxisListType.X)
                    # cast to bf16 with the 1/8 mean scale on ACT
                    nc.scalar.mul(xbarb_sb[:, q * QC:(q + 1) * QC, :],
                                  xbar_sb[:, q * QC:(q + 1) * QC, :], 0.125)
                    for c in range(q * QC, (q + 1) * QC):
                        for h in range(2):
                            nc.tensor.matmul(
                                (hT0_ps if h == 0 else hT1_ps)[:],
                                w1_sb[:, c * D + h * 128: c * D + h * 128 + 128],
                                xbarb_sb[:, c, :],
                                start=(c == 0), stop=(c == NVC - 1),
                            )
                # W2/b2 queued behind X/W1 on the same HW queue; they land
                # before the AllReduce completes.
                nc.sync.dma_start(w2_sb[:], W2B.ap())
                nc.sync.dma_start(b2_sb[:], B2.ap())

                # b1/8 columns (packed in W1B) -> fp32
                nc.vector.tensor_scalar_mul(
                    b1col_sb[:], w1_sb[:, NVC * D:NVC * D + 2], 0.125)
                # partial h^T + b1/8, cast to bf16 for the AllReduce
                nc.vector.tensor_scalar_add(
                    hT_sb[:, 0, :], hT0_ps[:], b1col_sb[:, 0:1])
                nc.vector.tensor_scalar_add(
                    hT_sb[:, 1, :], hT1_ps[:], b1col_sb[:, 1:2])

            # Preload the ACT Exp/Ln table during the AllReduce gap.
            nc.scalar.activation(dummy_sb[:], warm_sb[:],
                                 mybir.ActivationFunctionType.Exp)

            # AllReduce partial h^T across the 8 vocab shards (bf16, 32 KB).
            hb_in = dram.tile([128, 2 * B], BF16)
            hb_out = dram.tile([128, 2 * B], BF16, addr_space="Shared")
            nc.sync.dma_start(hb_in[:],
                              hT_sb[:].rearrange("p h b -> p (h b)"))
            nc.gpsimd.collective_compute(
                "AllReduce", mybir.AluOpType.add, replica_groups=rg,
                ins=[hb_in.opt()], outs=[hb_out.opt()])
            hsum_sb = work.tile([128, 2, B], BF16)
            nc.sync.dma_start(hsum_sb[:].rearrange("p h b -> p (h b)"),
                              hb_out[:])

            # Layer 2 + log-softmax.
            e_sb = work.tile([B, VS], F32)
            out_sb = work.tile([B, VS], F32)
            sumexp_sb = work.tile([B, 1], F32)
            sums8_sb = work.tile([B, NSPL], F32)

            with tc.tile_pool(name="ps3", bufs=1, space="PSUM") as ps3:
                logits_ps = ps3.tile([B, 4096], F32)      # 8 banks
                # 512-wide bank-aligned splits (416 tail) so each matmul's
                # accumulation group lives in a single PSUM bank.
                nsplits = [(k * 512, min(512, VS - k * 512)) for k in range(8)]
                # b2 streamed into each PSUM bank while PE idles in the
                # AllReduce gap; the h matmuls then accumulate onto it.
                for k, (n0, nw) in enumerate(nsplits):
                    nc.tensor.matmul(
                        logits_ps[:, n0:n0 + nw],
                        ones_sb[:],
                        b2_sb[:, n0:n0 + nw],
                        start=True, stop=False,
                    )
                for k, (n0, nw) in enumerate(nsplits):
                    for h in range(2):
                        nc.tensor.matmul(
                            logits_ps[:, n0:n0 + nw],
                            hsum_sb[:, h, :],
                            w2_sb[:, h, n0:n0 + nw],
                            start=False, stop=(h == 1),
                        )
                    # Per-bank exp so it overlaps the remaining layer-2
                    # matmuls; logits are O(+-3) so fp32 exp needs no
                    # max-subtraction.
                    nc.scalar.activation(
                        e_sb[:, n0:n0 + nw], logits_ps[:, n0:n0 + nw],
                        mybir.ActivationFunctionType.Exp,
                        accum_out=sums8_sb[:, k:k + 1])

                nc.vector.reduce_sum(sumexp_sb[:], sums8_sb[:],
                                     axis=mybir.AxisListType.X)

                # Global sumexp: AllGather the 8 per-core partial sums.
                # The [64]-across-partitions vector is stream-transposed onto
                # two partition rows so both collective DMAs are contiguous
                # bursts instead of 64 x 4B partition-strided descriptors.
                tr_in = work.tile([B, 32], F32)
                nc.vector.memset(tr_in[:], 0.0)
                nc.vector.tensor_copy(tr_in[:, 0:1], sumexp_sb[:])
                tr_out = work.tile([B, 32], F32)
                nc.vector.transpose(tr_out[:], tr_in[:])
                sb_in = dram.tile([2, 32], F32)
                sb_out = dram.tile([N_CORES, 2, 32], F32, addr_space="Shared")
                nc.sync.dma_start(sb_in[:], tr_out[0:B:32, :])
                nc.gpsimd.collective_compute(
                    "AllGather", mybir.AluOpType.bypass, replica_groups=rg,
                    ins=[sb_in.opt()], outs=[sb_out.opt()])
                sg_sb = work.tile([1, 2 * N_CORES * 32], F32)
                nc.sync.dma_start(sg_sb[:],
                                  sb_out[:].rearrange("r h b -> (r h b)"))
                stot_row = work.tile([1, B], F32)
                nc.vector.reduce_sum(
                    stot_row[:],
                    sg_sb[:].rearrange("p (r c) -> p c r", r=N_CORES),
                    axis=mybir.AxisListType.X)
                ln_row = work.tile([1, B], F32)
                nc.scalar.activation(ln_row[:], stot_row[:],
                                     mybir.ActivationFunctionType.Ln)
                ltr_in = work.tile([B, 32], F32)
                nc.vector.memset(ltr_in[:], 0.0)
                nc.vector.tensor_copy(ltr_in[0:1, :], ln_row[0:1, 0:32])
                nc.vector.tensor_copy(ltr_in[32:33, :], ln_row[0:1, 32:B])
                ltr_out = work.tile([B, 32], F32)
                nc.vector.transpose(ltr_out[:], ltr_in[:])
                logs_sb = work.tile([B, 1], F32)
                nc.vector.tensor_copy(logs_sb[:], ltr_out[:, 0:1])
                neglogs_sb = work.tile([B, 1], F32)
                nc.vector.tensor_scalar_mul(neglogs_sb[:], logs_sb[:], -1.0)

                # out = logits - log(sumexp): 4 chunks alternating DVE/ACT,
                # each chunk's store issued as soon as it is ready.
                CH = VS // 4
                for j in range(4):
                    c0 = j * CH
                    if j % 2 == 0:
                        nc.vector.tensor_scalar_sub(
                            out_sb[:, c0:c0 + CH], logits_ps[:, c0:c0 + CH],
                            logs_sb[:])
                    else:
                        nc.scalar.activation(
                            out_sb[:, c0:c0 + CH], logits_ps[:, c0:c0 + CH],
                            mybir.ActivationFunctionType.Identity,
                            bias=neglogs_sb[:])
                    nc.sync.dma_start(OUT.ap()[:, c0:c0 + CH],
                                      out_sb[:, c0:c0 + CH])

    nc.compile()
    return nc


def _get_nc():
    if "nc" not in _cache:
        _cache["nc"] = _build()
    return _cache["nc"]


def _make_in_maps(input_vec, W1, b1, W2, b2):
    import ml_dtypes
    BF = ml_dtypes.bfloat16

    input_vec = np.asarray(input_vec, dtype=np.float32)
    W1 = np.asarray(W1, dtype=np.float32)
    b1 = np.asarray(b1, dtype=np.float32)
    W2 = np.asarray(W2, dtype=np.float32)
    b2 = np.asarray(b2, dtype=np.float32)

    xr = input_vec.reshape(B, NCTX, V)
    in_maps = []
    for c in range(N_CORES):
        lo, hi = c * VS, (c + 1) * VS
        # X^T padded to 4096 v-rows, chunked: xt[p, ch, r] = X[r//8, r%8, lo+ch*128+p]
        xts = np.zeros((VP, ROWS), np.float32)
        xts[:VS] = xr[:, :, lo:hi].reshape(ROWS, VS).T
        xt = np.ascontiguousarray(
            xts.reshape(NVC, VC, ROWS).transpose(1, 0, 2)).astype(BF)
        # W1 chunks + b1/8 columns: w1b[p, ch*256 + h*128 + m] = W1[h*128+m, lo+ch*128+p]
        w1s = np.zeros((VP, D), np.float32)
        w1s[:VS] = W1[:, lo:hi].T
        w1b = np.zeros((VC, NVC * D + 2), np.float32)
        w1b[:, :NVC * D] = w1s.reshape(NVC, VC, D).transpose(1, 0, 2).reshape(VC, NVC * D)
        w1b[:, NVC * D:] = b1.reshape(2, 128).T
        # W2^T halves: w2b[p, h, n] = W2[lo+n, h*128+p]
        w2b = W2[lo:hi, :].T.reshape(2, 128, VS).transpose(1, 0, 2)
        in_maps.append({
            "xt": xt,
            "w1b": w1b.astype(BF),
            "w2b": np.ascontiguousarray(w2b).astype(BF),
            "b2": b2[None, lo:hi].astype(BF),
        })
    return in_maps


def kernel(input_vec, W1, b1, W2, b2, **_unused):
    in_maps = _make_in_maps(input_vec, W1, b1, W2, b2)
    _cache["in_maps"] = in_maps
    nc = _get_nc()
    res = run_bass_kernel_spmd(nc, in_maps, core_ids=list(range(N_CORES)))
    return np.concatenate([res.results[c]["out"] for c in range(N_CORES)],
                          axis=1)
